# revision 1
# baseline (speedup 1.0000x reference)
"""RWKV time-mix (WKV) kernel for 8 Trainium2 NeuronCores.

Strategy
--------
Data-parallel over B: each of the 8 cores gets 8 batches. Per core/batch,
everything runs in channel-major layout [C(part), T(free)]:

  host:   x^T (bf16), W^T (bf16), per-channel constants precomputed
  chip:   time-shift via free-dim slice, mixes on DVE (bf16),
          k/v/r projections on TensorE (bf16 -> f32 PSUM),
          WKV recurrence via DVE tensor_tensor_scan (f32):
              A_t = D*A_{t-1} + exp(k_t)*v_t     (scan, per channel)
              B_t = D*B_{t-1} + exp(k_t)
              y_t = (A_t + (e^u - 1)*EV_t) / (B_t + (e^u - 1)*E_t)
          division + sigmoid via exp/ln on ScalarE (single ACT table set):
              rwkv = num * exp(-(ln(den) + ln(1 + exp(-r))))
          output projection on TensorE, DMA out as out^T (f32)
  host:   concat + transpose back
"""

import contextlib
import ctypes
import os
import sys
import types

import numpy as np
import ml_dtypes


def _ensure_ntff_hook():
    """The image's antenv package lacks axon_hooks; provide it (and a
    working ctypes NTFF profile hook) so trace=True paths don't crash."""
    try:
        import antenv.axon_hooks  # noqa: F401
        return
    except ImportError:
        pass
    try:
        import antenv
    except ImportError:
        antenv = types.ModuleType("antenv")
        sys.modules["antenv"] = antenv
    mod = types.ModuleType("antenv.axon_hooks")
    _hook = [None]
    mod.set_axon_ntff_profile_hook = lambda h: _hook.__setitem__(0, h)
    mod.get_axon_ntff_profile_hook = lambda: _hook[0]
    sys.modules["antenv.axon_hooks"] = mod
    sys.modules["antenv"].axon_hooks = mod

    so_path = "/opt/axon/libaxon_pjrt.so"
    if os.path.exists(so_path):
        try:
            lib = ctypes.CDLL(so_path)
            if hasattr(lib, "axon_start_nrt_profile"):
                lib.axon_start_nrt_profile.argtypes = [
                    ctypes.POINTER(ctypes.c_int64), ctypes.c_size_t]
                lib.axon_start_nrt_profile.restype = ctypes.c_int64
                lib.axon_stop_nrt_profile.argtypes = [ctypes.c_char_p]
                lib.axon_stop_nrt_profile.restype = ctypes.c_int64

                @contextlib.contextmanager
                def _profile(output_dir, device_ids):
                    import jax
                    jax.devices()
                    if device_ids:
                        ids = (ctypes.c_int64 * len(device_ids))(*device_ids)
                        rc = lib.axon_start_nrt_profile(ids, len(device_ids))
                    else:
                        rc = lib.axon_start_nrt_profile(None, 0)
                    if rc != 0:
                        raise RuntimeError(f"axon_start_nrt_profile rc={rc}")
                    try:
                        yield
                    finally:
                        n = lib.axon_stop_nrt_profile(str(output_dir).encode())
                        print(f"profile: {n} file(s) written to {output_dir}",
                              file=sys.stderr)

                mod.set_axon_ntff_profile_hook(_profile)
        except OSError:
            pass


_ensure_ntff_hook()

import concourse.bass as bass
import concourse.mybir as mybir
import concourse.tile as tile
from concourse import bacc
from concourse.bass_utils import run_bass_kernel_spmd

B, T, C = 64, 512, 1024
NCORES = 8
BPC = B // NCORES          # batches per core
P = 128
CT = C // P                # channel tiles

F32 = mybir.dt.float32
BF16 = mybir.dt.bfloat16
AF = mybir.ActivationFunctionType
OP = mybir.AluOpType

_nc_cache = {}

# engine-placement toggles
NUMDEN_SPLIT = False      # num/den as ACT copy-scale + GpSimd add (else DVE STT)
SADD_ON_GPSIMD = False    # ln(den)+ln(1+e^-r) add on GpSimd
RWKV_ON_GPSIMD = False    # final num*f multiply on GpSimd
MIXADD_ON_GPSIMD = False  # mix tt_add on GpSimd
MIXMUL_ON_ACT = True      # mix dif*tm multiply on ScalarE (Copy w/ scale)


class _Bacc(bacc.Bacc):
    """Bacc whose ACT-table pass is pinned to the one set containing both
    exp and ln, so the Exp/Ln interleave doesn't thrash table loads."""

    def insert_act_table_loads(self):
        import concourse.mybir as mb
        from concourse.hw_specs import get_activation_tables
        from concourse.bacc import _bass_rust as br
        has_activation = any(
            isinstance(i, mb.InstActivation)
            for b in self.main_func.blocks
            for i in b.instructions
        )
        if not has_activation:
            return
        tables = []
        strip = {mb.ActivationFunctionType.Exp, mb.ActivationFunctionType.Ln}
        for name, fns in get_activation_tables(self.m.arch).items():
            if name != "natural_log_exp_and_others":
                fns = fns - strip
            tables.append((name, fns))
        br.insert_act_table_loads(self, tables)


def build_nc(debug=False):
    nc = _Bacc()
    dbg_names = ("xb", "xk", "E", "EV0", "A", "Bs", "num", "den", "er", "lnr", "f", "rw0")
    dbg = {}
    if debug:
        for n in dbg_names:
            dbg[n] = nc.declare_dram_parameter(f"dbg_{n}", [P, T], F32, isOutput=True)

    xt = nc.declare_dram_parameter("xt", [BPC, C, T], BF16, isOutput=False)
    wk = nc.declare_dram_parameter("wk", [C, C], BF16, isOutput=False)
    wv = nc.declare_dram_parameter("wv", [C, C], BF16, isOutput=False)
    wr = nc.declare_dram_parameter("wr", [C, C], BF16, isOutput=False)
    wo = nc.declare_dram_parameter("wo", [C, C], BF16, isOutput=False)
    # per-channel constants [P, CT, 5]: tmk, tmv, tmr, eu_m1, D
    cvec = nc.declare_dram_parameter("cvec", [P, CT, 5], F32, isOutput=False)
    out = nc.declare_dram_parameter("out", [BPC, C, T], F32, isOutput=True)

    with tile.TileContext(nc) as tc:
        with (
            tc.tile_pool(name="singles", bufs=1) as singles,
            tc.tile_pool(name="xbp", bufs=2) as xbp,
            tc.tile_pool(name="mixp", bufs=2) as mixp,
            tc.tile_pool(name="stagec", bufs=2) as stagec,
            tc.tile_pool(name="rwkvp", bufs=2) as rwkvp,
            tc.tile_pool(name="outp", bufs=3) as outp,
            tc.tile_pool(name="ps_kvr", bufs=2, space="PSUM") as ps_kvr,
            tc.tile_pool(name="ps_out", bufs=2, space="PSUM") as ps_out,
        ):
            # ---- one-time loads ----
            cv = singles.tile([P, CT, 5], F32, tag="cvec")
            nc.sync.dma_start(out=cv[:], in_=cvec[:])
            def emit_out_proj(b, rw):
                for dj in range(CT):
                    pso = ps_out.tile([P, T], F32, tag="pso", name="pso")
                    for kt in range(CT):
                        nc.tensor.matmul(
                            pso[:],
                            w_sb["o"][:, kt, dj * P:(dj + 1) * P],
                            rw[:, kt, :],
                            start=(kt == 0),
                            stop=(kt == CT - 1),
                        )
                    osb = outp.tile([P, T], F32, tag="osb", name="osb")
                    nc.scalar.copy(osb[:], pso[:])
                    nc.sync.dma_start(
                        out=out[b].rearrange("(ct p) t -> p ct t", p=P)[:, dj, :],
                        in_=osb[:],
                    )

            def load_xb(b):
                xb = xbp.tile([P, CT, T + 2], BF16, tag="xb", name="xb")
                nc.vector.memset(xb[:, :, 0:1], 0.0)
                src = xt[b].rearrange("(ct p) t -> p ct t", p=P)
                for ct in range(CT):
                    nc.sync.dma_start(out=xb[:, ct, 1:T + 1], in_=src[:, ct, :])
                return xb

            def make_mix_emitter(xb):
                # packed [P, CT, 3(k|v|r), T]; one broadcast add per c-tile
                mixall = mixp.tile([P, CT, 3, T], BF16, tag="mixall", name="mixall")
                mix = {name: mixall[:, :, ci] for ci, name in enumerate(("k", "v", "r"))}

                def emit_j(j):
                    x_ap = xb[:, j, 1:T + 1]
                    xx_ap = xb[:, j, 0:T]
                    dif = mixp.tile([P, T], BF16, tag="dif", name="dif")
                    nc.vector.tensor_tensor(dif[:], x_ap, xx_ap, OP.subtract)
                    if j % 2 == 0:
                        nc.vector.tensor_scalar_mul(mixall[:, j, 0, :], dif[:], cv[:, j, 0:1])
                        nc.vector.tensor_scalar_mul(mixall[:, j, 1, :], dif[:], cv[:, j, 1:2])
                        acts = (2,)
                    else:
                        nc.vector.tensor_scalar_mul(mixall[:, j, 0, :], dif[:], cv[:, j, 0:1])
                        acts = (1, 2)
                    for ci in acts:
                        nc.scalar.activation(mixall[:, j, ci, :], dif[:], AF.Copy,
                                             scale=cv[:, j, ci:ci + 1])
                    xx3 = xx_ap[:, None, :].to_broadcast((P, 3, T))
                    nc.vector.tensor_tensor(mixall[:, j], mixall[:, j], xx3, OP.add)
                return mix, emit_j

            def do_mixes(xb):
                mix, emit_j = make_mix_emitter(xb)
                for j in range(CT):
                    emit_j(j)
                return mix

            prev = None  # (b, rwkv tile) pending output projection
            xb_cur = load_xb(0)
            w_sb = {}
            for name, par in (("k", wk), ("v", wv), ("r", wr), ("o", wo)):
                t = singles.tile([P, CT, C], BF16, tag=f"w{name}", name=f"w{name}")
                src = par.rearrange("(ct p) d -> p ct d", p=P)
                for kt in range(CT):
                    nc.sync.dma_start(out=t[:, kt, :], in_=src[:, kt, :])
                w_sb[name] = t

            # D broadcast tiles for the scan multiplier
            Db = singles.tile([P, CT, T], F32, tag="Db")
            nc.vector.memset(Db[:], 1.0)
            for j in range(CT):
                nc.vector.tensor_scalar_mul(Db[:, j, :], Db[:, j, :], cv[:, j, 4:5])

            mix = do_mixes(xb_cur)
            for b in range(BPC):
                emit_mix_next = None
                if b + 1 < BPC:
                    xb_next = load_xb(b + 1)
                    mix_next, emit_mix_next = make_mix_emitter(xb_next)
                # ---- stage B+C: projections + WKV per channel-tile ----
                rw = rwkvp.tile([P, CT, T], BF16, tag="rwkv", name="rwkv")
                for j in range(CT):
                    psk = ps_kvr.tile([P, T], F32, tag="psk")
                    psv = ps_kvr.tile([P, T], F32, tag="psv")
                    psr = ps_kvr.tile([P, T], F32, tag="psr")
                    for nm, ps in (("k", psk), ("v", psv), ("r", psr)):
                        for kt in range(CT):
                            nc.tensor.matmul(
                                ps[:],
                                w_sb[nm][:, kt, j * P:(j + 1) * P],
                                mix[nm][:, kt, :],
                                start=(kt == 0),
                                stop=(kt == CT - 1),
                            )

                    eu = cv[:, j, 3:4]
                    E = stagec.tile([P, T], F32, tag="E")
                    nc.scalar.activation(E[:], psk[:], AF.Exp)
                    vsb = stagec.tile([P, T], BF16, tag="vsb", name="vsb")
                    nc.scalar.copy(vsb[:], psv[:])
                    EV = stagec.tile([P, T], F32, tag="EV")
                    nc.vector.tensor_tensor(EV[:], E[:], vsb[:], OP.mult)

                    # exclusive scans: A[:, t] = sum_{i<t} D^(t-1-i) EV_i, A[:, 0] = 0
                    A = stagec.tile([P, T], F32, tag="A")
                    nc.gpsimd.memset(A[:, 0:1], 0.0)
                    nc.vector.tensor_tensor_scan(
                        A[:, 1:T], Db[:, j, 0:T - 1], EV[:, 0:T - 1], 0.0, OP.mult, OP.add)
                    Bs = stagec.tile([P, T], F32, tag="Bs")
                    nc.gpsimd.memset(Bs[:, 0:1], 0.0)
                    nc.vector.tensor_tensor_scan(
                        Bs[:, 1:T], Db[:, j, 0:T - 1], E[:, 0:T - 1], 0.0, OP.mult, OP.add)

                    # num -> EV slot, den -> E slot (in-place)
                    numb = stagec.tile([P, T], BF16, tag="numb", name="numb")
                    nc.vector.scalar_tensor_tensor(numb[:], EV[:], eu, A[:], OP.mult, OP.add)
                    nc.vector.scalar_tensor_tensor(E[:], E[:], eu, Bs[:], OP.mult, OP.add)

                    # sigmoid(r): ln(1 + exp(-r)); division: ln(den)
                    er = stagec.tile([P, T], F32, tag="er")
                    nc.scalar.activation(er[:], psr[:], AF.Exp, scale=-1.0)
                    lnr = stagec.tile([P, T], F32, tag="lnr")
                    nc.scalar.activation(lnr[:], er[:], AF.Ln, bias=1.0)
                    ld = stagec.tile([P, T], F32, tag="ld")
                    nc.scalar.activation(ld[:], E[:], AF.Ln)
                    sadd_eng = nc.gpsimd if SADD_ON_GPSIMD else nc.vector
                    sadd_eng.tensor_tensor(lnr[:], lnr[:], ld[:], OP.add)
                    f = stagec.tile([P, T], BF16, tag="f", name="f")
                    nc.scalar.activation(f[:], lnr[:], AF.Exp, scale=-1.0)
                    nc.vector.tensor_tensor(rw[:, j, :], numb[:], f[:], OP.mult)

                    if emit_mix_next is not None:
                        emit_mix_next(j)

                    if debug and b == 0 and j == 0:
                        def dump(name, ap, cast=False):
                            if cast:
                                tmp = stagec.tile([P, T], F32, tag="dbgtmp", name="dbgtmp")
                                nc.vector.tensor_copy(tmp[:], ap)
                                ap = tmp[:]
                            nc.sync.dma_start(out=dbg[name][:], in_=ap)
                        dump("xb", xb_cur[:, 0, 1:T + 1], cast=True)
                        dump("xk", mix["k"][:, 0, :], cast=True)
                        dump("E", E[:])      # den after in-place
                        dump("EV0", EV[:])   # num after in-place
                        dump("A", A[:])
                        dump("Bs", Bs[:])
                        dump("num", EV[:])
                        dump("den", E[:])
                        dump("er", er[:])
                        dump("lnr", lnr[:])
                        dump("f", f[:])
                        dump("rw0", rw[:, 0, :], cast=True)

                if b + 1 < BPC:
                    mix = mix_next
                    xb_cur = xb_next
                if prev is not None:
                    emit_out_proj(*prev)
                prev = (b, rw)
            emit_out_proj(*prev)

    nc.compile()
    return nc


def _host_prep(x, time_decay, time_first, time_mix_k, time_mix_v, time_mix_r,
               Wk, Wv, Wr, Wo):
    bf = ml_dtypes.bfloat16
    f32 = np.float32
    xt = np.ascontiguousarray(x.transpose(0, 2, 1)).astype(bf)      # [B, C, T]
    wkt = np.ascontiguousarray(np.asarray(Wk, f32).T).astype(bf)    # [c, d]
    wvt = np.ascontiguousarray(np.asarray(Wv, f32).T).astype(bf)
    wrt = np.ascontiguousarray(np.asarray(Wr, f32).T).astype(bf)
    wot = np.ascontiguousarray(np.asarray(Wo, f32).T).astype(bf)

    D = np.exp(-np.exp(np.asarray(time_decay, f32))).astype(f32)
    eu_m1 = np.exp(np.asarray(time_first, f32)).astype(f32)  # e^u (col 3)
    tmk = np.asarray(time_mix_k, f32).reshape(C)
    tmv = np.asarray(time_mix_v, f32).reshape(C)
    tmr = np.asarray(time_mix_r, f32).reshape(C)
    cvec = np.stack([tmk, tmv, tmr, eu_m1, D], axis=-1)             # [C, 5]
    cvec = np.ascontiguousarray(cvec.reshape(CT, P, 5).transpose(1, 0, 2)).astype(f32)

    in_maps = []
    for i in range(NCORES):
        in_maps.append({
            "xt": xt[i * BPC:(i + 1) * BPC],
            "wk": wkt, "wv": wvt, "wr": wrt, "wo": wot,
            "cvec": cvec,
        })
    return in_maps


def kernel(x, time_decay, time_first, time_mix_k, time_mix_v, time_mix_r,
           Wk, Wv, Wr, Wo):
    x = np.asarray(x, np.float32)
    in_maps = _host_prep(x, time_decay, time_first, time_mix_k, time_mix_v,
                         time_mix_r, Wk, Wv, Wr, Wo)
    if "nc" not in _nc_cache:
        _nc_cache["nc"] = build_nc()
    res = run_bass_kernel_spmd(_nc_cache["nc"], in_maps, core_ids=list(range(NCORES)))
    _nc_cache["last_results"] = res
    full = np.concatenate([res.results[i]["out"] for i in range(NCORES)], axis=0)
    return np.ascontiguousarray(full.transpose(0, 2, 1))



# revision 3
# speedup vs baseline: 1.1229x; 1.1229x over previous
"""RWKV time-mix (WKV) kernel for 8 Trainium2 NeuronCores — v2.

Strategy
--------
Data-parallel over B: each of the 8 cores gets 8 batches, channel-major
layout [C(part), T(free)] on chip.

v2 changes vs v1:
  * All three time-mixes are computed on the HOST (they're cheap
    elementwise ops); xk/xr ship as fp8(e4m3), xv ships as bf16.
  * k and r projections run in fp8 with DoubleRow perf mode: 2 k-subtiles
    per PE pass -> half the matmul instructions of bf16. Weights are
    pre-scaled by 64 (and r's negated) on the host; the 1/64 un-scale is
    folded into the ACT exp scale.
  * k and (-r) accumulate into one 2-bank PSUM tile, so a single ACT
    Exp instruction produces both E = e^k and er = e^-r.
  * num/den are produced by ONE fused scalar_tensor_tensor over [P,2,T]
    (same e^u coefficient for both), placed on GpSimd.
  * ln(den) + ln(1+er) add placed on GpSimd; output copy stays on ACT.
  * Output returns as bf16 and is cast to f32 on the host.

WKV math per channel-tile j (all [128, T]):
    E = exp(k), er = exp(-r), EV = E*v
    A_t = sum_{i<t} D^{t-1-i} EV_i   (exclusive scan, f32)
    B_t = sum_{i<t} D^{t-1-i} E_i
    num = A + e^u*EV, den = B + e^u*E     (one fused STT, bf16 out)
    rwkv = num * exp(-(ln(den) + ln(1+er)))   [= sigmoid(r)*num/den]
"""

import contextlib
import ctypes
import os
import sys
import types

import numpy as np
import ml_dtypes


def _ensure_ntff_hook():
    """The image's antenv package lacks axon_hooks; provide it (and a
    working ctypes NTFF profile hook) so trace=True paths don't crash."""
    try:
        import antenv.axon_hooks  # noqa: F401
        return
    except ImportError:
        pass
    try:
        import antenv
    except ImportError:
        antenv = types.ModuleType("antenv")
        sys.modules["antenv"] = antenv
    mod = types.ModuleType("antenv.axon_hooks")
    _hook = [None]
    mod.set_axon_ntff_profile_hook = lambda h: _hook.__setitem__(0, h)
    mod.get_axon_ntff_profile_hook = lambda: _hook[0]
    sys.modules["antenv.axon_hooks"] = mod
    sys.modules["antenv"].axon_hooks = mod

    so_path = "/opt/axon/libaxon_pjrt.so"
    if os.path.exists(so_path):
        try:
            lib = ctypes.CDLL(so_path)
            if hasattr(lib, "axon_start_nrt_profile"):
                lib.axon_start_nrt_profile.argtypes = [
                    ctypes.POINTER(ctypes.c_int64), ctypes.c_size_t]
                lib.axon_start_nrt_profile.restype = ctypes.c_int64
                lib.axon_stop_nrt_profile.argtypes = [ctypes.c_char_p]
                lib.axon_stop_nrt_profile.restype = ctypes.c_int64

                @contextlib.contextmanager
                def _profile(output_dir, device_ids):
                    import jax
                    jax.devices()
                    if device_ids:
                        ids = (ctypes.c_int64 * len(device_ids))(*device_ids)
                        rc = lib.axon_start_nrt_profile(ids, len(device_ids))
                    else:
                        rc = lib.axon_start_nrt_profile(None, 0)
                    if rc != 0:
                        raise RuntimeError(f"axon_start_nrt_profile rc={rc}")
                    try:
                        yield
                    finally:
                        n = lib.axon_stop_nrt_profile(str(output_dir).encode())
                        print(f"profile: {n} file(s) written to {output_dir}",
                              file=sys.stderr)

                mod.set_axon_ntff_profile_hook(_profile)
        except OSError:
            pass


_ensure_ntff_hook()

import concourse.bass as bass
import concourse.mybir as mybir
import concourse.tile as tile
from concourse import bacc
from concourse.bass_utils import run_bass_kernel_spmd

B, T, C = 64, 512, 1024
NCORES = 8
BPC = B // NCORES          # batches per core
P = 128
CT = C // P                # channel tiles

F32 = mybir.dt.float32
BF16 = mybir.dt.bfloat16
F8 = mybir.dt.float8e4
AF = mybir.ActivationFunctionType
OP = mybir.AluOpType
DR = mybir.MatmulPerfMode.DoubleRow

WS = 64.0                  # fp8 weight pre-scale (un-scaled in ACT exp)

_nc_cache = {}

# engine-placement toggles
# NOTE: GpSimd (Pool) only supports plain TensorTensor/TensorScalar/memset —
# TensorScalarPtr (scalar_tensor_tensor, tensor_tensor_scan) fails the ISA
# engine check at codegen. PSUM is also unreachable from Pool.
ND_ON_GPSIMD = False       # fused num/den STT must stay on DVE
SADD_ON_GPSIMD = True      # ln(den)+ln(1+er) add on GpSimd (plain TT add)


class _Bacc(bacc.Bacc):
    """Bacc whose ACT-table pass is pinned to the one set containing both
    exp and ln, so the Exp/Ln interleave doesn't thrash table loads."""

    def insert_act_table_loads(self):
        import concourse.mybir as mb
        from concourse.hw_specs import get_activation_tables
        from concourse.bacc import _bass_rust as br
        has_activation = any(
            isinstance(i, mb.InstActivation)
            for b in self.main_func.blocks
            for i in b.instructions
        )
        if not has_activation:
            return
        tables = []
        strip = {mb.ActivationFunctionType.Exp, mb.ActivationFunctionType.Ln}
        for name, fns in get_activation_tables(self.m.arch).items():
            if name != "natural_log_exp_and_others":
                fns = fns - strip
            tables.append((name, fns))
        br.insert_act_table_loads(self, tables)


def build_nc():
    nc = _Bacc()

    xk8 = nc.declare_dram_parameter("xk8", [BPC, C, T], F8, isOutput=False)
    xr8 = nc.declare_dram_parameter("xr8", [BPC, C, T], F8, isOutput=False)
    xv = nc.declare_dram_parameter("xv", [BPC, C, T], BF16, isOutput=False)
    wk8 = nc.declare_dram_parameter("wk8", [C, C], F8, isOutput=False)
    wr8 = nc.declare_dram_parameter("wr8", [C, C], F8, isOutput=False)
    wv = nc.declare_dram_parameter("wv", [C, C], BF16, isOutput=False)
    wo = nc.declare_dram_parameter("wo", [C, C], BF16, isOutput=False)
    # per-channel constants [P, CT, 2]: e^u, D
    cvec = nc.declare_dram_parameter("cvec", [P, CT, 2], F32, isOutput=False)
    out = nc.declare_dram_parameter("out", [BPC, C, T], BF16, isOutput=True)

    with tile.TileContext(nc) as tc:
        with (
            tc.tile_pool(name="singles", bufs=1) as singles,
            tc.tile_pool(name="xp", bufs=2) as xp,
            tc.tile_pool(name="stage", bufs=2) as stage,
            tc.tile_pool(name="rwp", bufs=2) as rwp,
            tc.tile_pool(name="outp", bufs=3) as outp,
            tc.tile_pool(name="ps_kr", bufs=2, space="PSUM") as ps_kr,
            tc.tile_pool(name="ps_v", bufs=2, space="PSUM") as ps_v,
            tc.tile_pool(name="ps_o", bufs=2, space="PSUM") as ps_o,
        ):
            # ---- one-time loads ----
            cv = singles.tile([P, CT, 2], F32, tag="cvec")
            nc.sync.dma_start(out=cv[:], in_=cvec[:])

            w_sb = {}
            for name, par, dt in (("k", wk8, F8), ("r", wr8, F8),
                                  ("v", wv, BF16), ("o", wo, BF16)):
                t = singles.tile([P, CT, C], dt, tag=f"w{name}", name=f"w{name}")
                src = par.rearrange("(ct p) d -> p ct d", p=P)
                for kt in range(CT):
                    nc.sync.dma_start(out=t[:, kt, :], in_=src[:, kt, :])
                w_sb[name] = t

            # D broadcast tiles for the scan multiplier
            Db = singles.tile([P, CT, T], F32, tag="Db")
            nc.vector.memset(Db[:], 1.0)
            for j in range(CT):
                nc.vector.tensor_scalar_mul(Db[:, j, :], Db[:, j, :], cv[:, j, 1:2])

            def load_x(b):
                xkt = xp.tile([P, CT, T], F8, tag="xkt", name="xkt")
                xrt = xp.tile([P, CT, T], F8, tag="xrt", name="xrt")
                xvt = xp.tile([P, CT, T], BF16, tag="xvt", name="xvt")
                for par, t in ((xk8, xkt), (xr8, xrt), (xv, xvt)):
                    src = par[b].rearrange("(ct p) t -> p ct t", p=P)
                    for ct in range(CT):
                        nc.sync.dma_start(out=t[:, ct, :], in_=src[:, ct, :])
                return xkt, xrt, xvt

            def emit_out_proj(b, rw):
                for dj in range(CT):
                    pso = ps_o.tile([P, T], F32, tag="pso", name="pso")
                    for kt in range(CT):
                        nc.tensor.matmul(
                            pso[:],
                            w_sb["o"][:, kt, dj * P:(dj + 1) * P],
                            rw[:, kt, :],
                            start=(kt == 0),
                            stop=(kt == CT - 1),
                        )
                    osb = outp.tile([P, T], BF16, tag="osb", name="osb")
                    nc.scalar.copy(osb[:], pso[:])
                    nc.sync.dma_start(
                        out=out[b].rearrange("(ct p) t -> p ct t", p=P)[:, dj, :],
                        in_=osb[:],
                    )

            nd_eng = nc.gpsimd if ND_ON_GPSIMD else nc.vector
            sadd_eng = nc.gpsimd if SADD_ON_GPSIMD else nc.vector

            prev = None  # (b, rwkv tile) pending output projection
            x_cur = load_x(0)
            for b in range(BPC):
                if b + 1 < BPC:
                    x_next = load_x(b + 1)
                xkt, xrt, xvt = x_cur
                rw = rwp.tile([P, CT, T], BF16, tag="rwkv", name="rwkv")
                for j in range(CT):
                    # ---- projections: fp8 DoubleRow for -r (slot0), k (slot1) ----
                    ps = ps_kr.tile([P, 2, T], F32, tag="pskr", name="pskr")
                    for nm, slot, xt in (("r", 0, xrt), ("k", 1, xkt)):
                        for kk in range(0, CT, 2):
                            nc.tensor.matmul(
                                ps[:, slot, :],
                                w_sb[nm][:, kk:kk + 2, j * P:(j + 1) * P],
                                xt[:, kk:kk + 2, :],
                                start=(kk == 0),
                                stop=(kk == CT - 2),
                                perf_mode=DR,
                            )
                    pv = ps_v.tile([P, T], F32, tag="psv", name="psv")
                    for kt in range(CT):
                        nc.tensor.matmul(
                            pv[:],
                            w_sb["v"][:, kt, j * P:(j + 1) * P],
                            xvt[:, kt, :],
                            start=(kt == 0),
                            stop=(kt == CT - 1),
                        )

                    # ---- WKV ----
                    # Q = [er | E | EV]
                    Q = stage.tile([P, 3, T], F32, tag="Q", name="Q")
                    nc.scalar.activation(Q[:, 0:2, :], ps[:], AF.Exp, scale=1.0 / WS)
                    nc.vector.tensor_tensor(Q[:, 2, :], Q[:, 1, :], pv[:], OP.mult)

                    # AB = [B | A] exclusive scans
                    AB = stage.tile([P, 2, T], F32, tag="AB", name="AB")
                    nc.gpsimd.memset(AB[:, :, 0:1], 0.0)
                    nc.vector.tensor_tensor_scan(
                        AB[:, 0, 1:T], Db[:, j, 0:T - 1], Q[:, 1, 0:T - 1],
                        0.0, OP.mult, OP.add)
                    nc.vector.tensor_tensor_scan(
                        AB[:, 1, 1:T], Db[:, j, 0:T - 1], Q[:, 2, 0:T - 1],
                        0.0, OP.mult, OP.add)

                    # ND = [den | num] = (Q[:,1:3]*eu) + AB, bf16
                    ND = stage.tile([P, 2, T], BF16, tag="ND", name="ND")
                    nd_eng.scalar_tensor_tensor(
                        ND[:], Q[:, 1:3, :], cv[:, j, 0:1], AB[:],
                        OP.mult, OP.add)

                    # rwkv = num * exp(-(ln(den) + ln(1+er)))
                    lnr = stage.tile([P, T], F32, tag="lnr")
                    nc.scalar.activation(lnr[:], Q[:, 0, :], AF.Ln, bias=1.0)
                    ld = stage.tile([P, T], F32, tag="ld")
                    nc.scalar.activation(ld[:], ND[:, 0, :], AF.Ln)
                    sadd = stage.tile([P, T], F32, tag="sadd")
                    sadd_eng.tensor_tensor(sadd[:], lnr[:], ld[:], OP.add)
                    f = stage.tile([P, T], BF16, tag="f", name="f")
                    nc.scalar.activation(f[:], sadd[:], AF.Exp, scale=-1.0)
                    nc.vector.tensor_tensor(rw[:, j, :], ND[:, 1, :], f[:], OP.mult)

                if b + 1 < BPC:
                    x_cur = x_next
                if prev is not None:
                    emit_out_proj(*prev)
                prev = (b, rw)
            emit_out_proj(*prev)

    nc.compile()
    return nc


def _host_prep(x, time_decay, time_first, time_mix_k, time_mix_v, time_mix_r,
               Wk, Wv, Wr, Wo):
    bf = ml_dtypes.bfloat16
    f8 = ml_dtypes.float8_e4m3
    f32 = np.float32

    x = np.asarray(x, f32)
    xx = np.zeros_like(x)
    xx[:, 1:] = x[:, :-1]
    dif = x - xx
    tmk = np.asarray(time_mix_k, f32).reshape(1, 1, C)
    tmv = np.asarray(time_mix_v, f32).reshape(1, 1, C)
    tmr = np.asarray(time_mix_r, f32).reshape(1, 1, C)
    xk8 = np.ascontiguousarray((xx + tmk * dif).transpose(0, 2, 1)).astype(f8)
    xvb = np.ascontiguousarray((xx + tmv * dif).transpose(0, 2, 1)).astype(bf)
    xr8 = np.ascontiguousarray((xx + tmr * dif).transpose(0, 2, 1)).astype(f8)

    wk8 = np.ascontiguousarray(WS * np.asarray(Wk, f32).T).astype(f8)
    wr8 = np.ascontiguousarray(-WS * np.asarray(Wr, f32).T).astype(f8)
    wvt = np.ascontiguousarray(np.asarray(Wv, f32).T).astype(bf)
    wot = np.ascontiguousarray(np.asarray(Wo, f32).T).astype(bf)

    D = np.exp(-np.exp(np.asarray(time_decay, f32))).astype(f32)
    eu = np.exp(np.asarray(time_first, f32)).astype(f32)
    cvec = np.stack([eu, D], axis=-1)                               # [C, 2]
    cvec = np.ascontiguousarray(
        cvec.reshape(CT, P, 2).transpose(1, 0, 2)).astype(f32)

    in_maps = []
    for i in range(NCORES):
        sl = slice(i * BPC, (i + 1) * BPC)
        in_maps.append({
            "xk8": xk8[sl], "xr8": xr8[sl], "xv": xvb[sl],
            "wk8": wk8, "wr8": wr8, "wv": wvt, "wo": wot,
            "cvec": cvec,
        })
    return in_maps


def kernel(x, time_decay, time_first, time_mix_k, time_mix_v, time_mix_r,
           Wk, Wv, Wr, Wo):
    in_maps = _host_prep(x, time_decay, time_first, time_mix_k, time_mix_v,
                         time_mix_r, Wk, Wv, Wr, Wo)
    if "nc" not in _nc_cache:
        _nc_cache["nc"] = build_nc()
    res = run_bass_kernel_spmd(_nc_cache["nc"], in_maps, core_ids=list(range(NCORES)))
    _nc_cache["last_results"] = res
    full = np.concatenate(
        [np.asarray(res.results[i]["out"]) for i in range(NCORES)], axis=0)
    return np.ascontiguousarray(full.transpose(0, 2, 1)).astype(np.float32)


# revision 5
# speedup vs baseline: 1.2440x; 1.1078x over previous
"""RWKV time-mix (WKV) kernel for 8 Trainium2 NeuronCores — v2.

Strategy
--------
Data-parallel over B: each of the 8 cores gets 8 batches, channel-major
layout [C(part), T(free)] on chip.

v2 changes vs v1:
  * All three time-mixes are computed on the HOST (they're cheap
    elementwise ops); xk/xr ship as fp8(e4m3), xv ships as bf16.
  * k and r projections run in fp8 with DoubleRow perf mode: 2 k-subtiles
    per PE pass -> half the matmul instructions of bf16. Weights are
    pre-scaled by 64 (and r's negated) on the host; the 1/64 un-scale is
    folded into the ACT exp scale.
  * k and (-r) accumulate into one 2-bank PSUM tile, so a single ACT
    Exp instruction produces both E = e^k and er = e^-r.
  * num/den are produced by ONE fused scalar_tensor_tensor over [P,2,T]
    (same e^u coefficient for both), placed on GpSimd.
  * ln(den) + ln(1+er) add placed on GpSimd; output copy stays on ACT.
  * Output returns as bf16 and is cast to f32 on the host.

WKV math per channel-tile j (all [128, T]):
    E = exp(k), er = exp(-r), EV = E*v
    A_t = sum_{i<t} D^{t-1-i} EV_i   (exclusive scan, f32)
    B_t = sum_{i<t} D^{t-1-i} E_i
    num = A + e^u*EV, den = B + e^u*E     (one fused STT, bf16 out)
    rwkv = num * exp(-(ln(den) + ln(1+er)))   [= sigmoid(r)*num/den]
"""

import contextlib
import ctypes
import os
import sys
import types

import numpy as np
import ml_dtypes


def _ensure_ntff_hook():
    """The image's antenv package lacks axon_hooks; provide it (and a
    working ctypes NTFF profile hook) so trace=True paths don't crash."""
    try:
        import antenv.axon_hooks  # noqa: F401
        return
    except ImportError:
        pass
    try:
        import antenv
    except ImportError:
        antenv = types.ModuleType("antenv")
        sys.modules["antenv"] = antenv
    mod = types.ModuleType("antenv.axon_hooks")
    _hook = [None]
    mod.set_axon_ntff_profile_hook = lambda h: _hook.__setitem__(0, h)
    mod.get_axon_ntff_profile_hook = lambda: _hook[0]
    sys.modules["antenv.axon_hooks"] = mod
    sys.modules["antenv"].axon_hooks = mod

    so_path = "/opt/axon/libaxon_pjrt.so"
    if os.path.exists(so_path):
        try:
            lib = ctypes.CDLL(so_path)
            if hasattr(lib, "axon_start_nrt_profile"):
                lib.axon_start_nrt_profile.argtypes = [
                    ctypes.POINTER(ctypes.c_int64), ctypes.c_size_t]
                lib.axon_start_nrt_profile.restype = ctypes.c_int64
                lib.axon_stop_nrt_profile.argtypes = [ctypes.c_char_p]
                lib.axon_stop_nrt_profile.restype = ctypes.c_int64

                @contextlib.contextmanager
                def _profile(output_dir, device_ids):
                    import jax
                    jax.devices()
                    if device_ids:
                        ids = (ctypes.c_int64 * len(device_ids))(*device_ids)
                        rc = lib.axon_start_nrt_profile(ids, len(device_ids))
                    else:
                        rc = lib.axon_start_nrt_profile(None, 0)
                    if rc != 0:
                        raise RuntimeError(f"axon_start_nrt_profile rc={rc}")
                    try:
                        yield
                    finally:
                        n = lib.axon_stop_nrt_profile(str(output_dir).encode())
                        print(f"profile: {n} file(s) written to {output_dir}",
                              file=sys.stderr)

                mod.set_axon_ntff_profile_hook(_profile)
        except OSError:
            pass


_ensure_ntff_hook()

import concourse.bass as bass
import concourse.mybir as mybir
import concourse.tile as tile
from concourse import bacc
from concourse.bass_utils import run_bass_kernel_spmd

B, T, C = 64, 512, 1024
NCORES = 8
BPC = B // NCORES          # batches per core
P = 128
CT = C // P                # channel tiles

F32 = mybir.dt.float32
BF16 = mybir.dt.bfloat16
F8 = mybir.dt.float8e4
AF = mybir.ActivationFunctionType
OP = mybir.AluOpType
DR = mybir.MatmulPerfMode.DoubleRow

WS = 64.0                  # fp8 weight pre-scale (un-scaled in ACT exp)

_nc_cache = {}

# engine-placement toggles
# NOTE: GpSimd (Pool) only supports plain TensorTensor/TensorScalar/memset —
# TensorScalarPtr (scalar_tensor_tensor, tensor_tensor_scan) fails the ISA
# engine check at codegen. PSUM is also unreachable from Pool.
SADD_ON_GPSIMD = True      # ln(den)+ln(1+er) add on GpSimd (plain TT add)
SCAN_BF16 = True           # bf16 scan operands (attempt DVE 2x mode)
T2 = T + 2                 # padded free dim (scan/STT shift alignment)


class _Bacc(bacc.Bacc):
    """Bacc whose ACT-table pass is pinned to the one set containing both
    exp and ln, so the Exp/Ln interleave doesn't thrash table loads."""

    def insert_act_table_loads(self):
        import concourse.mybir as mb
        from concourse.hw_specs import get_activation_tables
        from concourse.bacc import _bass_rust as br
        has_activation = any(
            isinstance(i, mb.InstActivation)
            for b in self.main_func.blocks
            for i in b.instructions
        )
        if not has_activation:
            return
        tables = []
        strip = {mb.ActivationFunctionType.Exp, mb.ActivationFunctionType.Ln}
        for name, fns in get_activation_tables(self.m.arch).items():
            if name != "natural_log_exp_and_others":
                fns = fns - strip
            tables.append((name, fns))
        br.insert_act_table_loads(self, tables)


def build_nc():
    nc = _Bacc()

    xk8 = nc.declare_dram_parameter("xk8", [BPC, C, T], F8, isOutput=False)
    xr8 = nc.declare_dram_parameter("xr8", [BPC, C, T], F8, isOutput=False)
    xv = nc.declare_dram_parameter("xv", [BPC, C, T], BF16, isOutput=False)
    wk8 = nc.declare_dram_parameter("wk8", [C, C], F8, isOutput=False)
    wr8 = nc.declare_dram_parameter("wr8", [C, C], F8, isOutput=False)
    wv = nc.declare_dram_parameter("wv", [C, C], BF16, isOutput=False)
    wo = nc.declare_dram_parameter("wo", [C, C], BF16, isOutput=False)
    # per-channel constants [P, CT, 2]: e^u, D
    cvec = nc.declare_dram_parameter("cvec", [P, CT, 2], F32, isOutput=False)
    out = nc.declare_dram_parameter("out", [BPC, C, T], BF16, isOutput=True)

    SDT = BF16 if SCAN_BF16 else F32

    with tile.TileContext(nc) as tc:
        with (
            tc.tile_pool(name="singles", bufs=1) as singles,
            tc.tile_pool(name="xp", bufs=2) as xp,
            tc.tile_pool(name="stage", bufs=3) as stage,
            tc.tile_pool(name="rwp", bufs=2) as rwp,
            tc.tile_pool(name="outp", bufs=3) as outp,
            tc.tile_pool(name="ps_kr", bufs=2, space="PSUM") as ps_kr,
            tc.tile_pool(name="ps_v", bufs=2, space="PSUM") as ps_v,
            tc.tile_pool(name="ps_o", bufs=2, space="PSUM") as ps_o,
        ):
            # ---- one-time loads (x of batch 0/1 queued before bulky weights
            # so the first matmuls aren't stuck behind 6 MB of weight DMA) ----
            cv = singles.tile([P, CT, 2], F32, tag="cvec")
            nc.sync.dma_start(out=cv[:], in_=cvec[:])

            def load_x(b):
                xkt = xp.tile([P, CT, T], F8, tag="xkt", name="xkt")
                xrt = xp.tile([P, CT, T], F8, tag="xrt", name="xrt")
                xvt = xp.tile([P, CT, T], BF16, tag="xvt", name="xvt")
                for par, t in ((xk8, xkt), (xr8, xrt), (xv, xvt)):
                    src = par[b].rearrange("(ct p) t -> p ct t", p=P)
                    for ct in range(CT):
                        nc.sync.dma_start(out=t[:, ct, :], in_=src[:, ct, :])
                return xkt, xrt, xvt

            x_cur = load_x(0)

            w_sb = {}
            for name, par, dt in (("k", wk8, F8), ("r", wr8, F8),
                                  ("v", wv, BF16), ("o", wo, BF16)):
                t = singles.tile([P, CT, C], dt, tag=f"w{name}", name=f"w{name}")
                src = par.rearrange("(ct p) d -> p ct d", p=P)
                for kt in range(CT):
                    nc.sync.dma_start(out=t[:, kt, :], in_=src[:, kt, :])
                w_sb[name] = t

            # D broadcast tiles for the scan multiplier
            Db = singles.tile([P, CT, T], SDT, tag="Db")
            nc.vector.memset(Db[:], 1.0)
            for j in range(CT):
                nc.vector.tensor_scalar_mul(Db[:, j, :], Db[:, j, :], cv[:, j, 1:2])

            sadd_eng = nc.gpsimd if SADD_ON_GPSIMD else nc.vector

            def emit_oproj_group(b, rw, dj):
                pso = ps_o.tile([P, T], F32, tag="pso", name="pso")
                for kt in range(CT):
                    nc.tensor.matmul(
                        pso[:],
                        w_sb["o"][:, kt, dj * P:(dj + 1) * P],
                        rw[:, kt, :],
                        start=(kt == 0),
                        stop=(kt == CT - 1),
                    )
                osb = outp.tile([P, T], BF16, tag="osb", name="osb")
                nc.scalar.copy(osb[:], pso[:])
                nc.sync.dma_start(
                    out=out[b].rearrange("(ct p) t -> p ct t", p=P)[:, dj, :],
                    in_=osb[:],
                )

            def emit_head(xkt, xrt, xvt, rw, j):
                """Projections + exp + EV + scans + num/den for tile j.
                Returns refs needed by the (deferred) division tail."""
                # fp8 DoubleRow: -r into slot0, k into slot1 of 2-bank PSUM
                ps = ps_kr.tile([P, 2, T], F32, tag="pskr", name="pskr")
                for nm, slot, xt in (("r", 0, xrt), ("k", 1, xkt)):
                    for kk in range(0, CT, 2):
                        nc.tensor.matmul(
                            ps[:, slot, :],
                            w_sb[nm][:, kk:kk + 2, j * P:(j + 1) * P],
                            xt[:, kk:kk + 2, :],
                            start=(kk == 0),
                            stop=(kk == CT - 2),
                            perf_mode=DR,
                        )
                pv = ps_v.tile([P, T], F32, tag="psv", name="psv")
                for kt in range(CT):
                    nc.tensor.matmul(
                        pv[:],
                        w_sb["v"][:, kt, j * P:(j + 1) * P],
                        xvt[:, kt, :],
                        start=(kt == 0),
                        stop=(kt == CT - 1),
                    )

                # Q = [er | E | EV] at cols [2:T+2]
                Q = stage.tile([P, 3, T2], SDT, tag="Q", name="Q")
                nc.scalar.activation(Q[:, 0:2, 2:T + 2], ps[:], AF.Exp,
                                     scale=1.0 / WS)
                nc.vector.tensor_tensor(Q[:, 2, 2:T + 2], Q[:, 1, 2:T + 2],
                                        pv[:], OP.mult)

                # AB = [B | A]: INCLUSIVE scans at cols [2:T+2] (aligned for
                # the DVE 2x packed mode); col 1 zeroed = exclusive element 0.
                AB = stage.tile([P, 2, T2], SDT, tag="AB", name="AB")
                nc.gpsimd.memset(AB[:, :, 1:2], 0.0)
                nc.vector.tensor_tensor_scan(
                    AB[:, 0, 2:T + 2], Db[:, j, :], Q[:, 1, 2:T + 2],
                    0.0, OP.mult, OP.add)
                nc.vector.tensor_tensor_scan(
                    AB[:, 1, 2:T + 2], Db[:, j, :], Q[:, 2, 2:T + 2],
                    0.0, OP.mult, OP.add)

                # num/den = e^u * (EV|E) + exclusive(A|B) (shifted AB read)
                eu = cv[:, j, 0:1]
                numb = stage.tile([P, T], BF16, tag="numb", name="numb")
                nc.vector.scalar_tensor_tensor(
                    numb[:], Q[:, 2, 2:T + 2], eu, AB[:, 1, 1:T + 1],
                    OP.mult, OP.add)
                den = stage.tile([P, T], BF16, tag="den", name="den")
                nc.vector.scalar_tensor_tensor(
                    den[:], Q[:, 1, 2:T + 2], eu, AB[:, 0, 1:T + 1],
                    OP.mult, OP.add)
                return Q, numb, den

            def emit_tail(rw, j, Q, numb, den):
                """Division tail: rwkv = num * exp(-(ln(den)+ln(1+er)))."""
                lnr = stage.tile([P, T], F32, tag="lnr")
                nc.scalar.activation(lnr[:], Q[:, 0, 2:T + 2], AF.Ln, bias=1.0)
                ld = stage.tile([P, T], F32, tag="ld")
                nc.scalar.activation(ld[:], den[:], AF.Ln)
                sadd = stage.tile([P, T], F32, tag="sadd")
                sadd_eng.tensor_tensor(sadd[:], lnr[:], ld[:], OP.add)
                f = stage.tile([P, T], BF16, tag="f", name="f")
                nc.scalar.activation(f[:], sadd[:], AF.Exp, scale=-1.0)
                nc.vector.tensor_tensor(rw[:, j, :], numb[:], f[:], OP.mult)

            pending_tail = None   # (rw, j, Q, numb, den)
            pending_oproj = None  # (b, rw) whose groups drip out per-j
            for b in range(BPC):
                xkt, xrt, xvt = x_cur
                if b + 1 < BPC:
                    x_cur = load_x(b + 1)
                rw = rwp.tile([P, CT, T], BF16, tag="rwkv", name="rwkv")
                for j in range(CT):
                    head = emit_head(xkt, xrt, xvt, rw, j)
                    if pending_tail is not None:
                        emit_tail(*pending_tail)
                    pending_tail = (rw, j) + head
                    if pending_oproj is not None:
                        emit_oproj_group(*pending_oproj, j)
                pending_oproj = (b, rw)
            emit_tail(*pending_tail)
            for dj in range(CT):
                emit_oproj_group(*pending_oproj, dj)

    nc.compile()
    return nc


def _host_prep(x, time_decay, time_first, time_mix_k, time_mix_v, time_mix_r,
               Wk, Wv, Wr, Wo):
    bf = ml_dtypes.bfloat16
    f8 = ml_dtypes.float8_e4m3
    f32 = np.float32

    x = np.asarray(x, f32)
    xx = np.zeros_like(x)
    xx[:, 1:] = x[:, :-1]
    dif = x - xx
    tmk = np.asarray(time_mix_k, f32).reshape(1, 1, C)
    tmv = np.asarray(time_mix_v, f32).reshape(1, 1, C)
    tmr = np.asarray(time_mix_r, f32).reshape(1, 1, C)
    xk8 = np.ascontiguousarray((xx + tmk * dif).transpose(0, 2, 1)).astype(f8)
    xvb = np.ascontiguousarray((xx + tmv * dif).transpose(0, 2, 1)).astype(bf)
    xr8 = np.ascontiguousarray((xx + tmr * dif).transpose(0, 2, 1)).astype(f8)

    wk8 = np.ascontiguousarray(WS * np.asarray(Wk, f32).T).astype(f8)
    wr8 = np.ascontiguousarray(-WS * np.asarray(Wr, f32).T).astype(f8)
    wvt = np.ascontiguousarray(np.asarray(Wv, f32).T).astype(bf)
    wot = np.ascontiguousarray(np.asarray(Wo, f32).T).astype(bf)

    D = np.exp(-np.exp(np.asarray(time_decay, f32))).astype(f32)
    eu = np.exp(np.asarray(time_first, f32)).astype(f32)
    cvec = np.stack([eu, D], axis=-1)                               # [C, 2]
    cvec = np.ascontiguousarray(
        cvec.reshape(CT, P, 2).transpose(1, 0, 2)).astype(f32)

    in_maps = []
    for i in range(NCORES):
        sl = slice(i * BPC, (i + 1) * BPC)
        in_maps.append({
            "xk8": xk8[sl], "xr8": xr8[sl], "xv": xvb[sl],
            "wk8": wk8, "wr8": wr8, "wv": wvt, "wo": wot,
            "cvec": cvec,
        })
    return in_maps


def kernel(x, time_decay, time_first, time_mix_k, time_mix_v, time_mix_r,
           Wk, Wv, Wr, Wo):
    in_maps = _host_prep(x, time_decay, time_first, time_mix_k, time_mix_v,
                         time_mix_r, Wk, Wv, Wr, Wo)
    if "nc" not in _nc_cache:
        _nc_cache["nc"] = build_nc()
    res = run_bass_kernel_spmd(_nc_cache["nc"], in_maps, core_ids=list(range(NCORES)))
    _nc_cache["last_results"] = res
    full = np.concatenate(
        [np.asarray(res.results[i]["out"]) for i in range(NCORES)], axis=0)
    return np.ascontiguousarray(full.transpose(0, 2, 1)).astype(np.float32)


# revision 7
# speedup vs baseline: 1.2522x; 1.0066x over previous
"""RWKV time-mix (WKV) kernel for 8 Trainium2 NeuronCores — v2.

Strategy
--------
Data-parallel over B: each of the 8 cores gets 8 batches, channel-major
layout [C(part), T(free)] on chip.

v2 changes vs v1:
  * All three time-mixes are computed on the HOST (they're cheap
    elementwise ops); xk/xr ship as fp8(e4m3), xv ships as bf16.
  * k and r projections run in fp8 with DoubleRow perf mode: 2 k-subtiles
    per PE pass -> half the matmul instructions of bf16. Weights are
    pre-scaled by 64 (and r's negated) on the host; the 1/64 un-scale is
    folded into the ACT exp scale.
  * k and (-r) accumulate into one 2-bank PSUM tile, so a single ACT
    Exp instruction produces both E = e^k and er = e^-r.
  * num/den are produced by ONE fused scalar_tensor_tensor over [P,2,T]
    (same e^u coefficient for both), placed on GpSimd.
  * ln(den) + ln(1+er) add placed on GpSimd; output copy stays on ACT.
  * Output returns as bf16 and is cast to f32 on the host.

WKV math per channel-tile j (all [128, T]):
    E = exp(k), er = exp(-r), EV = E*v
    A_t = sum_{i<t} D^{t-1-i} EV_i   (exclusive scan, f32)
    B_t = sum_{i<t} D^{t-1-i} E_i
    num = A + e^u*EV, den = B + e^u*E     (one fused STT, bf16 out)
    rwkv = num * exp(-(ln(den) + ln(1+er)))   [= sigmoid(r)*num/den]
"""

import contextlib
import ctypes
import os
import sys
import types

import numpy as np
import ml_dtypes


def _ensure_ntff_hook():
    """The image's antenv package lacks axon_hooks; provide it (and a
    working ctypes NTFF profile hook) so trace=True paths don't crash."""
    try:
        import antenv.axon_hooks  # noqa: F401
        return
    except ImportError:
        pass
    try:
        import antenv
    except ImportError:
        antenv = types.ModuleType("antenv")
        sys.modules["antenv"] = antenv
    mod = types.ModuleType("antenv.axon_hooks")
    _hook = [None]
    mod.set_axon_ntff_profile_hook = lambda h: _hook.__setitem__(0, h)
    mod.get_axon_ntff_profile_hook = lambda: _hook[0]
    sys.modules["antenv.axon_hooks"] = mod
    sys.modules["antenv"].axon_hooks = mod

    so_path = "/opt/axon/libaxon_pjrt.so"
    if os.path.exists(so_path):
        try:
            lib = ctypes.CDLL(so_path)
            if hasattr(lib, "axon_start_nrt_profile"):
                lib.axon_start_nrt_profile.argtypes = [
                    ctypes.POINTER(ctypes.c_int64), ctypes.c_size_t]
                lib.axon_start_nrt_profile.restype = ctypes.c_int64
                lib.axon_stop_nrt_profile.argtypes = [ctypes.c_char_p]
                lib.axon_stop_nrt_profile.restype = ctypes.c_int64

                @contextlib.contextmanager
                def _profile(output_dir, device_ids):
                    import jax
                    jax.devices()
                    if device_ids:
                        ids = (ctypes.c_int64 * len(device_ids))(*device_ids)
                        rc = lib.axon_start_nrt_profile(ids, len(device_ids))
                    else:
                        rc = lib.axon_start_nrt_profile(None, 0)
                    if rc != 0:
                        raise RuntimeError(f"axon_start_nrt_profile rc={rc}")
                    try:
                        yield
                    finally:
                        n = lib.axon_stop_nrt_profile(str(output_dir).encode())
                        print(f"profile: {n} file(s) written to {output_dir}",
                              file=sys.stderr)

                mod.set_axon_ntff_profile_hook(_profile)
        except OSError:
            pass


_ensure_ntff_hook()

import concourse.bass as bass
import concourse.mybir as mybir
import concourse.tile as tile
from concourse import bacc
from concourse.bass_utils import run_bass_kernel_spmd

B, T, C = 64, 512, 1024
NCORES = 8
BPC = B // NCORES          # batches per core
P = 128
CT = C // P                # channel tiles

F32 = mybir.dt.float32
BF16 = mybir.dt.bfloat16
F8 = mybir.dt.float8e4
AF = mybir.ActivationFunctionType
OP = mybir.AluOpType
DR = mybir.MatmulPerfMode.DoubleRow

WS = 64.0                  # fp8 weight pre-scale (un-scaled in ACT exp)

_nc_cache = {}

# engine-placement toggles
# NOTE: GpSimd (Pool) only supports plain TensorTensor/TensorScalar/memset —
# TensorScalarPtr (scalar_tensor_tensor, tensor_tensor_scan) fails the ISA
# engine check at codegen. PSUM is also unreachable from Pool.
SADD_ON_GPSIMD = True      # ln(den)+ln(1+er) add on GpSimd (plain TT add)
SCAN_BF16 = False          # bf16 scans measured no 2x on HW (scan is ~2.2
                           # cyc/elem regardless) and slowed EV; keep f32
T2 = T + 2                 # padded free dim (scan/STT shift alignment)


class _Bacc(bacc.Bacc):
    """Bacc whose ACT-table pass is pinned to the one set containing both
    exp and ln, so the Exp/Ln interleave doesn't thrash table loads."""

    def insert_act_table_loads(self):
        import concourse.mybir as mb
        from concourse.hw_specs import get_activation_tables
        from concourse.bacc import _bass_rust as br
        has_activation = any(
            isinstance(i, mb.InstActivation)
            for b in self.main_func.blocks
            for i in b.instructions
        )
        if not has_activation:
            return
        tables = []
        strip = {mb.ActivationFunctionType.Exp, mb.ActivationFunctionType.Ln}
        for name, fns in get_activation_tables(self.m.arch).items():
            if name != "natural_log_exp_and_others":
                fns = fns - strip
            tables.append((name, fns))
        br.insert_act_table_loads(self, tables)


def build_nc():
    nc = _Bacc()

    xk8 = nc.declare_dram_parameter("xk8", [BPC, C, T], F8, isOutput=False)
    xr8 = nc.declare_dram_parameter("xr8", [BPC, C, T], F8, isOutput=False)
    xv = nc.declare_dram_parameter("xv", [BPC, C, T], BF16, isOutput=False)
    wk8 = nc.declare_dram_parameter("wk8", [C, C], F8, isOutput=False)
    wr8 = nc.declare_dram_parameter("wr8", [C, C], F8, isOutput=False)
    wv = nc.declare_dram_parameter("wv", [C, C], BF16, isOutput=False)
    wo = nc.declare_dram_parameter("wo", [C, C], BF16, isOutput=False)
    # per-channel constants [P, CT, 2]: e^u, D
    cvec = nc.declare_dram_parameter("cvec", [P, CT, 2], F32, isOutput=False)
    out = nc.declare_dram_parameter("out", [BPC, C, T], BF16, isOutput=True)

    SDT = BF16 if SCAN_BF16 else F32

    with tile.TileContext(nc) as tc:
        with (
            tc.tile_pool(name="singles", bufs=1) as singles,
            tc.tile_pool(name="xp", bufs=2) as xp,
            tc.tile_pool(name="stage", bufs=3) as stage,
            tc.tile_pool(name="rwp", bufs=2) as rwp,
            tc.tile_pool(name="outp", bufs=3) as outp,
            tc.tile_pool(name="ps_kr", bufs=2, space="PSUM") as ps_kr,
            tc.tile_pool(name="ps_v", bufs=2, space="PSUM") as ps_v,
            tc.tile_pool(name="ps_o", bufs=2, space="PSUM") as ps_o,
        ):
            # ---- one-time loads (x of batch 0/1 queued before bulky weights
            # so the first matmuls aren't stuck behind 6 MB of weight DMA) ----
            cv = singles.tile([P, CT, 2], F32, tag="cvec")
            nc.sync.dma_start(out=cv[:], in_=cvec[:])

            def load_x(b):
                xkt = xp.tile([P, CT, T], F8, tag="xkt", name="xkt")
                xrt = xp.tile([P, CT, T], F8, tag="xrt", name="xrt")
                xvt = xp.tile([P, CT, T], BF16, tag="xvt", name="xvt")
                for par, t in ((xk8, xkt), (xr8, xrt), (xv, xvt)):
                    src = par[b].rearrange("(ct p) t -> p ct t", p=P)
                    for ct in range(CT):
                        nc.sync.dma_start(out=t[:, ct, :], in_=src[:, ct, :])
                return xkt, xrt, xvt

            x_cur = load_x(0)

            w_sb = {}
            for name, par, dt in (("k", wk8, F8), ("r", wr8, F8),
                                  ("v", wv, BF16), ("o", wo, BF16)):
                t = singles.tile([P, CT, C], dt, tag=f"w{name}", name=f"w{name}")
                src = par.rearrange("(ct p) d -> p ct d", p=P)
                for kt in range(CT):
                    nc.sync.dma_start(out=t[:, kt, :], in_=src[:, kt, :])
                w_sb[name] = t

            # D broadcast tiles for the scan multiplier
            Db = singles.tile([P, CT, T], SDT, tag="Db")
            nc.vector.memset(Db[:], 1.0)
            for j in range(CT):
                nc.vector.tensor_scalar_mul(Db[:, j, :], Db[:, j, :], cv[:, j, 1:2])

            sadd_eng = nc.gpsimd if SADD_ON_GPSIMD else nc.vector

            def emit_oproj_group(b, rw, dj):
                pso = ps_o.tile([P, T], F32, tag="pso", name="pso")
                for kt in range(CT):
                    nc.tensor.matmul(
                        pso[:],
                        w_sb["o"][:, kt, dj * P:(dj + 1) * P],
                        rw[:, kt, :],
                        start=(kt == 0),
                        stop=(kt == CT - 1),
                    )
                osb = outp.tile([P, T], BF16, tag="osb", name="osb")
                nc.scalar.copy(osb[:], pso[:])
                nc.sync.dma_start(
                    out=out[b].rearrange("(ct p) t -> p ct t", p=P)[:, dj, :],
                    in_=osb[:],
                )

            def emit_head(xkt, xrt, xvt, rw, j):
                """Projections + exp + EV + scans + num/den for tile j.
                Returns refs needed by the (deferred) division tail."""
                # fp8 DoubleRow: -r into slot0, k into slot1 of 2-bank PSUM
                ps = ps_kr.tile([P, 2, T], F32, tag="pskr", name="pskr")
                for nm, slot, xt in (("r", 0, xrt), ("k", 1, xkt)):
                    for kk in range(0, CT, 2):
                        nc.tensor.matmul(
                            ps[:, slot, :],
                            w_sb[nm][:, kk:kk + 2, j * P:(j + 1) * P],
                            xt[:, kk:kk + 2, :],
                            start=(kk == 0),
                            stop=(kk == CT - 2),
                            perf_mode=DR,
                        )
                pv = ps_v.tile([P, T], F32, tag="psv", name="psv")
                for kt in range(CT):
                    nc.tensor.matmul(
                        pv[:],
                        w_sb["v"][:, kt, j * P:(j + 1) * P],
                        xvt[:, kt, :],
                        start=(kt == 0),
                        stop=(kt == CT - 1),
                    )

                # Q = [er | E | EV] at cols [2:T+2]
                Q = stage.tile([P, 3, T2], SDT, tag="Q", name="Q")
                nc.scalar.activation(Q[:, 0:2, 2:T + 2], ps[:], AF.Exp,
                                     scale=1.0 / WS)
                nc.vector.tensor_tensor(Q[:, 2, 2:T + 2], Q[:, 1, 2:T + 2],
                                        pv[:], OP.mult)

                # AB = [B | A]: INCLUSIVE scans at cols [2:T+2] (aligned for
                # the DVE 2x packed mode); col 1 zeroed = exclusive element 0.
                AB = stage.tile([P, 2, T2], SDT, tag="AB", name="AB")
                nc.gpsimd.memset(AB[:, :, 1:2], 0.0)
                nc.vector.tensor_tensor_scan(
                    AB[:, 0, 2:T + 2], Db[:, j, :], Q[:, 1, 2:T + 2],
                    0.0, OP.mult, OP.add)
                nc.vector.tensor_tensor_scan(
                    AB[:, 1, 2:T + 2], Db[:, j, :], Q[:, 2, 2:T + 2],
                    0.0, OP.mult, OP.add)

                # num/den = e^u * (EV|E) + exclusive(A|B) (shifted AB read)
                eu = cv[:, j, 0:1]
                numb = stage.tile([P, T], BF16, tag="numb", name="numb")
                nc.vector.scalar_tensor_tensor(
                    numb[:], Q[:, 2, 2:T + 2], eu, AB[:, 1, 1:T + 1],
                    OP.mult, OP.add)
                den = stage.tile([P, T], BF16, tag="den", name="den")
                nc.vector.scalar_tensor_tensor(
                    den[:], Q[:, 1, 2:T + 2], eu, AB[:, 0, 1:T + 1],
                    OP.mult, OP.add)
                return Q, numb, den

            def emit_tail(rw, j, Q, numb, den):
                """Division tail: rwkv = num * exp(-(ln(den)+ln(1+er)))."""
                lnr = stage.tile([P, T], F32, tag="lnr")
                nc.scalar.activation(lnr[:], Q[:, 0, 2:T + 2], AF.Ln, bias=1.0)
                ld = stage.tile([P, T], F32, tag="ld")
                nc.scalar.activation(ld[:], den[:], AF.Ln)
                sadd = stage.tile([P, T], F32, tag="sadd")
                sadd_eng.tensor_tensor(sadd[:], lnr[:], ld[:], OP.add)
                f = stage.tile([P, T], BF16, tag="f", name="f")
                nc.scalar.activation(f[:], sadd[:], AF.Exp, scale=-1.0)
                nc.vector.tensor_tensor(rw[:, j, :], numb[:], f[:], OP.mult)

            pending_tail = None   # (rw, j, Q, numb, den)
            pending_oproj = None  # (b, rw) whose groups drip out per-j
            for b in range(BPC):
                xkt, xrt, xvt = x_cur
                if b + 1 < BPC:
                    x_cur = load_x(b + 1)
                rw = rwp.tile([P, CT, T], BF16, tag="rwkv", name="rwkv")
                for j in range(CT):
                    head = emit_head(xkt, xrt, xvt, rw, j)
                    if pending_tail is not None:
                        emit_tail(*pending_tail)
                    pending_tail = (rw, j) + head
                    # o-proj groups of the previous batch drip out one tile
                    # late (dj = j-1) so the final rwkv of that batch has a
                    # full tile of slack before group 0 needs it.
                    if pending_oproj is not None and j >= 1:
                        emit_oproj_group(*pending_oproj, j - 1)
                        if j == CT - 1:
                            emit_oproj_group(*pending_oproj, j)
                pending_oproj = (b, rw)
            emit_tail(*pending_tail)
            for dj in range(CT):
                emit_oproj_group(*pending_oproj, dj)

    nc.compile()
    return nc


def _host_prep(x, time_decay, time_first, time_mix_k, time_mix_v, time_mix_r,
               Wk, Wv, Wr, Wo):
    bf = ml_dtypes.bfloat16
    f8 = ml_dtypes.float8_e4m3
    f32 = np.float32

    x = np.asarray(x, f32)
    xx = np.zeros_like(x)
    xx[:, 1:] = x[:, :-1]
    dif = x - xx
    tmk = np.asarray(time_mix_k, f32).reshape(1, 1, C)
    tmv = np.asarray(time_mix_v, f32).reshape(1, 1, C)
    tmr = np.asarray(time_mix_r, f32).reshape(1, 1, C)
    xk8 = np.ascontiguousarray((xx + tmk * dif).transpose(0, 2, 1)).astype(f8)
    xvb = np.ascontiguousarray((xx + tmv * dif).transpose(0, 2, 1)).astype(bf)
    xr8 = np.ascontiguousarray((xx + tmr * dif).transpose(0, 2, 1)).astype(f8)

    wk8 = np.ascontiguousarray(WS * np.asarray(Wk, f32).T).astype(f8)
    wr8 = np.ascontiguousarray(-WS * np.asarray(Wr, f32).T).astype(f8)
    wvt = np.ascontiguousarray(np.asarray(Wv, f32).T).astype(bf)
    wot = np.ascontiguousarray(np.asarray(Wo, f32).T).astype(bf)

    D = np.exp(-np.exp(np.asarray(time_decay, f32))).astype(f32)
    eu = np.exp(np.asarray(time_first, f32)).astype(f32)
    cvec = np.stack([eu, D], axis=-1)                               # [C, 2]
    cvec = np.ascontiguousarray(
        cvec.reshape(CT, P, 2).transpose(1, 0, 2)).astype(f32)

    in_maps = []
    for i in range(NCORES):
        sl = slice(i * BPC, (i + 1) * BPC)
        in_maps.append({
            "xk8": xk8[sl], "xr8": xr8[sl], "xv": xvb[sl],
            "wk8": wk8, "wr8": wr8, "wv": wvt, "wo": wot,
            "cvec": cvec,
        })
    return in_maps


def kernel(x, time_decay, time_first, time_mix_k, time_mix_v, time_mix_r,
           Wk, Wv, Wr, Wo):
    in_maps = _host_prep(x, time_decay, time_first, time_mix_k, time_mix_v,
                         time_mix_r, Wk, Wv, Wr, Wo)
    if "nc" not in _nc_cache:
        _nc_cache["nc"] = build_nc()
    res = run_bass_kernel_spmd(_nc_cache["nc"], in_maps, core_ids=list(range(NCORES)))
    _nc_cache["last_results"] = res
    full = np.concatenate(
        [np.asarray(res.results[i]["out"]) for i in range(NCORES)], axis=0)
    return np.ascontiguousarray(full.transpose(0, 2, 1)).astype(np.float32)


# revision 11
# speedup vs baseline: 1.3691x; 1.0934x over previous
"""RWKV time-mix (WKV) kernel for 8 Trainium2 NeuronCores — v2.

Strategy
--------
Data-parallel over B: each of the 8 cores gets 8 batches, channel-major
layout [C(part), T(free)] on chip.

v2 changes vs v1:
  * All three time-mixes are computed on the HOST (they're cheap
    elementwise ops); xk/xr ship as fp8(e4m3), xv ships as bf16.
  * k and r projections run in fp8 with DoubleRow perf mode: 2 k-subtiles
    per PE pass -> half the matmul instructions of bf16. Weights are
    pre-scaled by 64 (and r's negated) on the host; the 1/64 un-scale is
    folded into the ACT exp scale.
  * k and (-r) accumulate into one 2-bank PSUM tile, so a single ACT
    Exp instruction produces both E = e^k and er = e^-r.
  * The sigmoid is folded into the denominator: rwkv = num/(den*(1+er));
    den2 = (er+1)*den is ONE DVE STT -- no ln(1+er), no extra add.
  * E/er/EV/v all bf16 so the EV multiply runs in the DVE 2x packed mode;
    scans keep f32 multiplier/state (D in bf16 would distort the decay).
  * Output returns as bf16 and is cast to f32 on the host.

WKV math per channel-tile j (all [128, T]):
    E = exp(k), er = exp(-r), EV = E*v
    A_t = sum_{i<t} D^{t-1-i} EV_i   (exclusive scan, f32 state)
    B_t = sum_{i<t} D^{t-1-i} E_i
    num = A + e^u*EV, den = B + e^u*E     (separate STTs)
    rwkv = num * exp(-ln((er+1)*den))     [= sigmoid(r)*num/den]
"""

import contextlib
import ctypes
import os
import sys
import types

import numpy as np
import ml_dtypes


def _ensure_ntff_hook():
    """The image's antenv package lacks axon_hooks; provide it (and a
    working ctypes NTFF profile hook) so trace=True paths don't crash."""
    try:
        import antenv.axon_hooks  # noqa: F401
        return
    except ImportError:
        pass
    try:
        import antenv
    except ImportError:
        antenv = types.ModuleType("antenv")
        sys.modules["antenv"] = antenv
    mod = types.ModuleType("antenv.axon_hooks")
    _hook = [None]
    mod.set_axon_ntff_profile_hook = lambda h: _hook.__setitem__(0, h)
    mod.get_axon_ntff_profile_hook = lambda: _hook[0]
    sys.modules["antenv.axon_hooks"] = mod
    sys.modules["antenv"].axon_hooks = mod

    so_path = "/opt/axon/libaxon_pjrt.so"
    if os.path.exists(so_path):
        try:
            lib = ctypes.CDLL(so_path)
            if hasattr(lib, "axon_start_nrt_profile"):
                lib.axon_start_nrt_profile.argtypes = [
                    ctypes.POINTER(ctypes.c_int64), ctypes.c_size_t]
                lib.axon_start_nrt_profile.restype = ctypes.c_int64
                lib.axon_stop_nrt_profile.argtypes = [ctypes.c_char_p]
                lib.axon_stop_nrt_profile.restype = ctypes.c_int64

                @contextlib.contextmanager
                def _profile(output_dir, device_ids):
                    import jax
                    jax.devices()
                    if device_ids:
                        ids = (ctypes.c_int64 * len(device_ids))(*device_ids)
                        rc = lib.axon_start_nrt_profile(ids, len(device_ids))
                    else:
                        rc = lib.axon_start_nrt_profile(None, 0)
                    if rc != 0:
                        raise RuntimeError(f"axon_start_nrt_profile rc={rc}")
                    try:
                        yield
                    finally:
                        n = lib.axon_stop_nrt_profile(str(output_dir).encode())
                        print(f"profile: {n} file(s) written to {output_dir}",
                              file=sys.stderr)

                mod.set_axon_ntff_profile_hook(_profile)
        except OSError:
            pass


_ensure_ntff_hook()

import concourse.bass as bass
import concourse.mybir as mybir
import concourse.tile as tile
from concourse import bacc
from concourse.bass_utils import run_bass_kernel_spmd

B, T, C = 64, 512, 1024
NCORES = 8
BPC = B // NCORES          # batches per core
P = 128
CT = C // P                # channel tiles

F32 = mybir.dt.float32
BF16 = mybir.dt.bfloat16
F8 = mybir.dt.float8e4
AF = mybir.ActivationFunctionType
OP = mybir.AluOpType
DR = mybir.MatmulPerfMode.DoubleRow

WS = 64.0                  # fp8 weight pre-scale (un-scaled in ACT exp)

_nc_cache = {}

# engine-placement toggles
# NOTE: GpSimd (Pool) only supports plain TensorTensor/TensorScalar/memset —
# TensorScalarPtr (scalar_tensor_tensor, tensor_tensor_scan) fails the ISA
# engine check at codegen. PSUM is also unreachable from Pool.
T2 = T + 2                 # padded free dim (scan/STT shift alignment)


class _Bacc(bacc.Bacc):
    """Bacc whose ACT-table pass is pinned to the one set containing both
    exp and ln, so the Exp/Ln interleave doesn't thrash table loads."""

    def insert_act_table_loads(self):
        import concourse.mybir as mb
        from concourse.hw_specs import get_activation_tables
        from concourse.bacc import _bass_rust as br
        has_activation = any(
            isinstance(i, mb.InstActivation)
            for b in self.main_func.blocks
            for i in b.instructions
        )
        if not has_activation:
            return
        tables = []
        strip = {mb.ActivationFunctionType.Exp, mb.ActivationFunctionType.Ln}
        for name, fns in get_activation_tables(self.m.arch).items():
            if name != "natural_log_exp_and_others":
                fns = fns - strip
            tables.append((name, fns))
        br.insert_act_table_loads(self, tables)


def build_nc():
    nc = _Bacc()

    xk8 = nc.declare_dram_parameter("xk8", [BPC, C, T], F8, isOutput=False)
    xr8 = nc.declare_dram_parameter("xr8", [BPC, C, T], F8, isOutput=False)
    xv = nc.declare_dram_parameter("xv", [BPC, C, T], BF16, isOutput=False)
    wk8 = nc.declare_dram_parameter("wk8", [C, C], F8, isOutput=False)
    wr8 = nc.declare_dram_parameter("wr8", [C, C], F8, isOutput=False)
    wv = nc.declare_dram_parameter("wv", [C, C], BF16, isOutput=False)
    wo = nc.declare_dram_parameter("wo", [C, C], BF16, isOutput=False)
    # per-channel constants [P, CT, 2]: e^u, D
    cvec = nc.declare_dram_parameter("cvec", [P, CT, 2], F32, isOutput=False)
    out = nc.declare_dram_parameter("out", [BPC, C, T], BF16, isOutput=True)

    with tile.TileContext(nc) as tc:
        with (
            tc.tile_pool(name="singles", bufs=1) as singles,
            tc.tile_pool(name="xp", bufs=2) as xp,
            tc.tile_pool(name="stage", bufs=3) as stage,
            tc.tile_pool(name="rwp", bufs=2) as rwp,
            tc.tile_pool(name="outp", bufs=3) as outp,
            tc.tile_pool(name="ps_kr", bufs=2, space="PSUM") as ps_kr,
            tc.tile_pool(name="ps_v", bufs=2, space="PSUM") as ps_v,
            tc.tile_pool(name="ps_o", bufs=2, space="PSUM") as ps_o,
        ):
            # ---- one-time loads. DMA queue order = dependency order of the
            # first matmuls: xr8(0)+wr8 first (r-projection leads each tile),
            # then xk8(0)+wk8, xv(0)+wv; wo last (first needed ~50us in). ----
            cv = singles.tile([P, CT, 2], F32, tag="cvec")
            nc.sync.dma_start(out=cv[:], in_=cvec[:])

            def _load_w(name, par, dt):
                t = singles.tile([P, CT, C], dt, tag=f"w{name}", name=f"w{name}")
                src = par.rearrange("(ct p) d -> p ct d", p=P)
                for kt in range(CT):
                    nc.sync.dma_start(out=t[:, kt, :], in_=src[:, kt, :])
                return t

            def _load_xpart(par, t, b):
                src = par[b].rearrange("(ct p) t -> p ct t", p=P)
                for ct in range(CT):
                    nc.sync.dma_start(out=t[:, ct, :], in_=src[:, ct, :])

            def load_x(b, order=("r", "k", "v")):
                tiles = {
                    "k": xp.tile([P, CT, T], F8, tag="xkt", name="xkt"),
                    "r": xp.tile([P, CT, T], F8, tag="xrt", name="xrt"),
                    "v": xp.tile([P, CT, T], BF16, tag="xvt", name="xvt"),
                }
                pars = {"k": xk8, "r": xr8, "v": xv}
                for nm in order:
                    _load_xpart(pars[nm], tiles[nm], b)
                return tiles["k"], tiles["r"], tiles["v"]

            w_sb = {}
            x0 = {}
            x0["r"] = xp.tile([P, CT, T], F8, tag="xrt", name="xrt")
            _load_xpart(xr8, x0["r"], 0)
            w_sb["r"] = _load_w("r", wr8, F8)
            x0["k"] = xp.tile([P, CT, T], F8, tag="xkt", name="xkt")
            _load_xpart(xk8, x0["k"], 0)
            w_sb["k"] = _load_w("k", wk8, F8)
            x0["v"] = xp.tile([P, CT, T], BF16, tag="xvt", name="xvt")
            _load_xpart(xv, x0["v"], 0)
            w_sb["v"] = _load_w("v", wv, BF16)
            w_sb["o"] = _load_w("o", wo, BF16)
            x_cur = (x0["k"], x0["r"], x0["v"])

            # D broadcast tiles for the scan multiplier
            Db = singles.tile([P, CT, T], F32, tag="Db")
            nc.vector.memset(Db[:], 1.0)
            for j in range(CT):
                nc.vector.tensor_scalar_mul(Db[:, j, :], Db[:, j, :], cv[:, j, 1:2])

            def emit_oproj_group(b, rw, dj):
                pso = ps_o.tile([P, T], F32, tag="pso", name="pso")
                for kt in range(CT):
                    nc.tensor.matmul(
                        pso[:],
                        w_sb["o"][:, kt, dj * P:(dj + 1) * P],
                        rw[:, kt, :],
                        start=(kt == 0),
                        stop=(kt == CT - 1),
                    )
                osb = outp.tile([P, T], BF16, tag="osb", name="osb")
                nc.scalar.copy(osb[:], pso[:])
                nc.sync.dma_start(
                    out=out[b].rearrange("(ct p) t -> p ct t", p=P)[:, dj, :],
                    in_=osb[:],
                )

            def emit_head(xkt, xrt, xvt, rw, j):
                """Projections + exp + EV + scans + num/den for tile j.
                Returns refs needed by the (deferred) division tail."""
                # fp8 DoubleRow: -r into slot0, k into slot1 of 2-bank PSUM
                ps = ps_kr.tile([P, 2, T], F32, tag="pskr", name="pskr")
                for nm, slot, xt in (("r", 0, xrt), ("k", 1, xkt)):
                    for kk in range(0, CT, 2):
                        nc.tensor.matmul(
                            ps[:, slot, :],
                            w_sb[nm][:, kk:kk + 2, j * P:(j + 1) * P],
                            xt[:, kk:kk + 2, :],
                            start=(kk == 0),
                            stop=(kk == CT - 2),
                            perf_mode=DR,
                        )
                pv = ps_v.tile([P, T], F32, tag="psv", name="psv")
                for kt in range(CT):
                    nc.tensor.matmul(
                        pv[:],
                        w_sb["v"][:, kt, j * P:(j + 1) * P],
                        xvt[:, kt, :],
                        start=(kt == 0),
                        stop=(kt == CT - 1),
                    )

                # Q = [er | E | EV] at cols [2:T+2], bf16 (EV mult gets DVE 2x)
                Q = stage.tile([P, 3, T2], BF16, tag="Q", name="Q")
                nc.scalar.activation(Q[:, 0:2, 2:T + 2], ps[:], AF.Exp,
                                     scale=1.0 / WS)
                vsb = stage.tile([P, T], BF16, tag="vsb", name="vsb")
                nc.scalar.copy(vsb[:], pv[:])
                nc.vector.tensor_tensor(Q[:, 2, 2:T + 2], Q[:, 1, 2:T + 2],
                                        vsb[:], OP.mult)

                # AB = [B | A]: INCLUSIVE scans (f32 state/output) at cols
                # [2:T+2]; col 1 zeroed = exclusive element 0.
                AB = stage.tile([P, 2, T2], F32, tag="AB", name="AB")
                nc.gpsimd.memset(AB[:, :, 1:2], 0.0)
                nc.vector.tensor_tensor_scan(
                    AB[:, 0, 2:T + 2], Db[:, j, :], Q[:, 1, 2:T + 2],
                    0.0, OP.mult, OP.add)
                nc.vector.tensor_tensor_scan(
                    AB[:, 1, 2:T + 2], Db[:, j, :], Q[:, 2, 2:T + 2],
                    0.0, OP.mult, OP.add)

                # num/den = e^u * (EV|E) + exclusive(A|B) (shifted AB read)
                eu = cv[:, j, 0:1]
                numb = stage.tile([P, T], BF16, tag="numb", name="numb")
                nc.vector.scalar_tensor_tensor(
                    numb[:], Q[:, 2, 2:T + 2], eu, AB[:, 1, 1:T + 1],
                    OP.mult, OP.add)
                den = stage.tile([P, T], F32, tag="den", name="den")
                nc.vector.scalar_tensor_tensor(
                    den[:], Q[:, 1, 2:T + 2], eu, AB[:, 0, 1:T + 1],
                    OP.mult, OP.add)
                # den2 = (er + 1) * den  [sigmoid folded into denominator]
                den2 = stage.tile([P, T], BF16, tag="den2", name="den2")
                nc.vector.scalar_tensor_tensor(
                    den2[:], Q[:, 0, 2:T + 2], 1.0, den[:], OP.add, OP.mult)
                return numb, den2

            def emit_tail(rw, j, numb, den2):
                """Division tail: rwkv = num * exp(-ln(den2))."""
                ld = stage.tile([P, T], F32, tag="ld")
                nc.scalar.activation(ld[:], den2[:], AF.Ln)
                f = stage.tile([P, T], BF16, tag="f", name="f")
                nc.scalar.activation(f[:], ld[:], AF.Exp, scale=-1.0)
                nc.vector.tensor_tensor(rw[:, j, :], numb[:], f[:], OP.mult)

            pending_tail = None   # (rw, j, Q, numb, den)
            pending_oproj = None  # (b, rw) whose groups drip out per-j
            for b in range(BPC):
                xkt, xrt, xvt = x_cur
                if b + 1 < BPC:
                    x_cur = load_x(b + 1)
                rw = rwp.tile([P, CT, T], BF16, tag="rwkv", name="rwkv")
                for j in range(CT):
                    head = emit_head(xkt, xrt, xvt, rw, j)
                    if pending_tail is not None:
                        emit_tail(*pending_tail)
                    pending_tail = (rw, j) + head
                    # o-proj groups of the previous batch drip out one tile
                    # late (dj = j-1) so the final rwkv of that batch has a
                    # full tile of slack before group 0 needs it.
                    if pending_oproj is not None and j >= 1:
                        emit_oproj_group(*pending_oproj, j - 1)
                        if j == CT - 1:
                            emit_oproj_group(*pending_oproj, j)
                pending_oproj = (b, rw)
            emit_tail(*pending_tail)
            for dj in range(CT):
                emit_oproj_group(*pending_oproj, dj)

    nc.compile()
    return nc


def _host_prep(x, time_decay, time_first, time_mix_k, time_mix_v, time_mix_r,
               Wk, Wv, Wr, Wo):
    bf = ml_dtypes.bfloat16
    f8 = ml_dtypes.float8_e4m3
    f32 = np.float32

    x = np.asarray(x, f32)
    xx = np.zeros_like(x)
    xx[:, 1:] = x[:, :-1]
    dif = x - xx
    tmk = np.asarray(time_mix_k, f32).reshape(1, 1, C)
    tmv = np.asarray(time_mix_v, f32).reshape(1, 1, C)
    tmr = np.asarray(time_mix_r, f32).reshape(1, 1, C)
    xk8 = np.ascontiguousarray((xx + tmk * dif).transpose(0, 2, 1)).astype(f8)
    xvb = np.ascontiguousarray((xx + tmv * dif).transpose(0, 2, 1)).astype(bf)
    xr8 = np.ascontiguousarray((xx + tmr * dif).transpose(0, 2, 1)).astype(f8)

    wk8 = np.ascontiguousarray(WS * np.asarray(Wk, f32).T).astype(f8)
    wr8 = np.ascontiguousarray(-WS * np.asarray(Wr, f32).T).astype(f8)
    wvt = np.ascontiguousarray(np.asarray(Wv, f32).T).astype(bf)
    wot = np.ascontiguousarray(np.asarray(Wo, f32).T).astype(bf)

    D = np.exp(-np.exp(np.asarray(time_decay, f32))).astype(f32)
    eu = np.exp(np.asarray(time_first, f32)).astype(f32)
    cvec = np.stack([eu, D], axis=-1)                               # [C, 2]
    cvec = np.ascontiguousarray(
        cvec.reshape(CT, P, 2).transpose(1, 0, 2)).astype(f32)

    in_maps = []
    for i in range(NCORES):
        sl = slice(i * BPC, (i + 1) * BPC)
        in_maps.append({
            "xk8": xk8[sl], "xr8": xr8[sl], "xv": xvb[sl],
            "wk8": wk8, "wr8": wr8, "wv": wvt, "wo": wot,
            "cvec": cvec,
        })
    return in_maps


def kernel(x, time_decay, time_first, time_mix_k, time_mix_v, time_mix_r,
           Wk, Wv, Wr, Wo):
    in_maps = _host_prep(x, time_decay, time_first, time_mix_k, time_mix_v,
                         time_mix_r, Wk, Wv, Wr, Wo)
    if "nc" not in _nc_cache:
        _nc_cache["nc"] = build_nc()
    res = run_bass_kernel_spmd(_nc_cache["nc"], in_maps, core_ids=list(range(NCORES)))
    _nc_cache["last_results"] = res
    full = np.concatenate(
        [np.asarray(res.results[i]["out"]) for i in range(NCORES)], axis=0)
    return np.ascontiguousarray(full.transpose(0, 2, 1)).astype(np.float32)
